# revision 13
# baseline (speedup 1.0000x reference)
"""AttentionPooling Trainium2 kernel (v2: t-pair layout).

Math (per batch row b):
    x   = target[b] + hist[b]              # [T, D]
    h   = relu(x @ W + Wb)                 # [T, D]
    lg  = h @ q (+ q_bias, softmax-invariant -> ignored)
    s   = softmax(lg)                      # over T
    out = sum_t s_t * hist[b, t]           # [D]

Device strategy (pure data parallel over batch across 8 cores):
  - hist loaded HBM->SBUF with fp32->bf16 cast (SWDGE) into a "t-pair"
    layout: partition q = t//2 (100 partitions), free = (b, t%2, d).
    Each DMA descriptor moves 1024B (read) -> 512B (write), contiguous:
    half the descriptors of a [t, (b,d)] layout and no sub-512B write
    penalty.  One DMA per 64-batch iteration.
  - PE transposes [100, 128] blocks -> xT [d, (b, par, tlo)]; the
    PSUM->SBUF copy is fused with the broadcast add of targetT.
  - Main matmul: H^T = W^T @ xT (bf16, W stationary), relu+bias on ACT
    over 1024-col PSUM chunks.
  - q-matmul with q replicated 32x, 4 row-strips of 400 cols (2 b's per
    strip) -> one [128, 400] PSUM tile per 8 b's; exp on ACT.
  - w transposed back (PE, [128,100] blocks) to [tlo, strip-dups] for
    use as pooling stationary.
  - Pooling per (b, parity): w-chunk stationary [100, 32] @ hist chunk
    [100, 128], PSUM-accumulated over parity.  Denominators sum_t w via
    a ones-column moving operand, 4 small matmuls per group.
  - Final normalize (divide by sum_t w) + layout de-permute on host.
"""

import sys

sys.path.insert(0, "/opt/trn_rl_repo")

import numpy as np

import concourse.bacc as bacc
import concourse.bass as bass
import concourse.mybir as mybir
import concourse.tile as tile
from concourse import masks
from concourse.bass_utils import run_bass_kernel_spmd

F32 = mybir.dt.float32
BF16 = mybir.dt.bfloat16
AF = mybir.ActivationFunctionType

NCORES = 8
B, T, D = 16384, 200, 128
BC = B // NCORES          # 2048 batch rows per core
TP = T // 2               # 100 t-pair partitions
B_IT = 64                 # batch rows per outer iteration
NGRP = B_IT // 8          # 8 groups of 8 b's (q/exp/wt/pool)
OUTW = NGRP * 258         # 2064 out cols per strip per iter


def build(nc, b_core=BC, dbg=False):
    nit = b_core // B_IT
    hist = nc.dram_tensor("hist", [b_core, T, D], F32, kind="ExternalInput")
    tgt = nc.dram_tensor("target", [b_core, D], F32, kind="ExternalInput")
    w_in = nc.dram_tensor("W", [D, D], F32, kind="ExternalInput")
    wb_in = nc.dram_tensor("Wb", [D], F32, kind="ExternalInput")
    q_in = nc.dram_tensor("q", [D, 1], F32, kind="ExternalInput")
    out_dev = nc.dram_tensor("out_dev", [nit, 4, OUTW], F32, kind="ExternalOutput")
    if dbg:
        dbg_nt = nc.dram_tensor("dbg_nt", [TP, B_IT * 256], F32, kind="ExternalOutput")
        dbg_ht = nc.dram_tensor("dbg_ht", [128, B_IT * T], F32, kind="ExternalOutput")
        dbg_hh = nc.dram_tensor("dbg_hh", [128, B_IT * T], F32, kind="ExternalOutput")
        dbg_w = nc.dram_tensor("dbg_w", [128, 400], F32, kind="ExternalOutput")
        dbg_wt = nc.dram_tensor("dbg_wt", [TP, 512], F32, kind="ExternalOutput")
        dbg_out = nc.dram_tensor("dbg_out", [128, OUTW], F32, kind="ExternalOutput")

    from contextlib import ExitStack
    with tile.TileContext(nc) as tc, ExitStack() as es:
        consts = es.enter_context(tc.tile_pool(name="consts", bufs=1))
        nt_pool = es.enter_context(tc.tile_pool(name="nt", bufs=CFG["nt"]))
        ht_pool = es.enter_context(tc.tile_pool(name="ht", bufs=CFG["ht"]))
        h_pool = es.enter_context(tc.tile_pool(name="h", bufs=CFG["hh"]))
        w_pool = es.enter_context(tc.tile_pool(name="w", bufs=4))
        wt_pool = es.enter_context(tc.tile_pool(name="wt", bufs=CFG["wtb"]))
        out_pool = es.enter_context(tc.tile_pool(name="out", bufs=CFG["outt"]))
        ps_tp = es.enter_context(tc.tile_pool(name="ps_tp", bufs=CFG["tp"], space="PSUM"))
        ps_mm = es.enter_context(tc.tile_pool(name="ps_mm", bufs=CFG["mm"], space="PSUM"))
        ps_q = es.enter_context(tc.tile_pool(name="ps_q", bufs=CFG["q"], space="PSUM"))
        ps_wt = es.enter_context(tc.tile_pool(name="ps_wt", bufs=CFG["wt"], space="PSUM"))
        ps_pool = es.enter_context(tc.tile_pool(name="ps_pool", bufs=CFG["pool"], space="PSUM"))

        # ---- constants ----
        ident = consts.tile([128, 128], BF16)
        masks.make_identity(nc, ident[:, :])

        w_f32 = consts.tile([D, D], F32)
        nc.sync.dma_start(out=w_f32, in_=w_in.ap())
        w_bf = consts.tile([D, D], BF16)
        nc.vector.tensor_copy(out=w_bf, in_=w_f32)

        wbias = consts.tile([D, 1], F32)
        nc.sync.dma_start(out=wbias, in_=wb_in.ap()[:, None])

        q_f32 = consts.tile([D, 1], F32)
        nc.sync.dma_start(out=q_f32, in_=q_in.ap())
        q_bf = consts.tile([D, 1], BF16)
        nc.vector.tensor_copy(out=q_bf, in_=q_f32)
        q32 = consts.tile([D, 32], BF16)
        nc.vector.tensor_copy(
            out=q32,
            in_=bass.AP(tensor=q_bf.tensor, offset=q_bf.offset,
                        ap=[q_bf.ap[0], [0, 32]]),
        )

        ones1 = consts.tile([TP, 1], BF16)
        nc.vector.memset(ones1, 1.0)

        # targetT [d, b_core] bf16
        tgtT = consts.tile([D, b_core], BF16)
        for k in range((b_core + 127) // 128):
            bn = min(128, b_core - k * 128)
            t_f32 = w_pool.tile([128, D], F32, tag="tsetup")
            nc.sync.dma_start(out=t_f32[0:bn], in_=tgt.ap()[k * 128:k * 128 + bn, :])
            t_bf = w_pool.tile([128, D], BF16, tag="tsetup_bf")
            nc.vector.tensor_copy(out=t_bf[0:bn], in_=t_f32[0:bn])
            tpp = ps_tp.tile([128, 800], BF16, tag="tp")
            nc.tensor.transpose(tpp[:, 0:bn], t_bf[0:bn], ident[0:bn, 0:bn])
            nc.vector.tensor_copy(out=tgtT[:, k * 128:k * 128 + bn], in_=tpp[:, 0:bn])

        # ---- main loop ----
        for it in range(nit):
            b0 = it * B_IT

            # t-pair tile: [q=100, (b, par*d = 256)] bf16, cast in DMA
            nt = nt_pool.tile([TP, B_IT * 256], BF16, tag="nt")
            ntv = nt.rearrange("q (b x) -> q b x", x=256)
            src = hist.ap()[b0:b0 + B_IT]
            nc.gpsimd.dma_start(
                out=ntv,
                in_=bass.AP(tensor=src.tensor, offset=src.offset,
                            ap=[[2 * D, TP], [T * D, B_IT], [1, 2 * D]]),
            )
            nt4 = nt.rearrange("q (b p d) -> q b p d", p=2, d=D)

            # target expanded 4x for a packed-innermost broadcast AP
            tgx4 = w_pool.tile([128, B_IT * 4], BF16, tag="tgx4")
            sl = tgtT[:, b0:b0 + B_IT]
            nc.vector.tensor_copy(
                out=tgx4,
                in_=bass.AP(tensor=sl.tensor, offset=sl.offset,
                            ap=[sl.ap[0], sl.ap[1], [0, 4]]),
            )
            tgx4v = tgx4.rearrange("d (b r) -> d b r", r=4)

            # transposes -> xT [d, (b, par, tlo)] with fused target add; the
            # add views each b's 200 cols as (50, 4) chunks so every operand
            # has a packed innermost dim (enables the DVE 2x bf16 mode)
            ht = ht_pool.tile([128, B_IT * T], BF16, tag="ht")
            htv = ht.rearrange("d (b p t) -> d b p t", p=2, t=TP)
            for g in range(B_IT // 4) if "tp" not in SKIP else []:
                j0 = 4 * g
                tp = ps_tp.tile([128, 800], BF16, tag="tp")
                tpv = tp.rearrange("d (b p t) -> d b p t", p=2, t=TP)
                for jj in range(4):
                    for par in range(2):
                        nc.tensor.transpose(
                            tpv[:, jj, par, :], nt4[:, j0 + jj, par, :],
                            ident[0:TP, 0:TP])
                tg4 = tgx4v[:, j0:j0 + 4, :]
                nc.vector.tensor_add(
                    ht.rearrange("d (b c x) -> d b c x",
                                 c=50, x=4)[:, j0:j0 + 4, :, :],
                    tp.rearrange("d (b c x) -> d b c x", c=50, x=4),
                    bass.AP(tensor=tg4.tensor, offset=tg4.offset,
                            ap=[tg4.ap[0], tg4.ap[1], [0, 50], tg4.ap[2]]),
                )

            # H^T = relu(W^T xT + bias)  [e, (b, par, tlo)]
            hh = h_pool.tile([128, B_IT * T], BF16, tag="hh")
            for k in range(13) if "mm" not in SKIP else []:
                w_cols = 1024 if k < 12 else 512
                mm = ps_mm.tile([128, 1024], F32)
                for h in range(w_cols // 512):
                    c = 1024 * k + 512 * h
                    nc.tensor.matmul(mm[:, 512 * h:512 * h + 512], w_bf,
                                     ht[:, c:c + 512], start=True, stop=True)
                nc.scalar.activation(hh[:, 1024 * k:1024 * k + w_cols],
                                     mm[:, 0:w_cols], AF.Relu, bias=wbias)

            # per group of 8 b's: q-matmul, exp, w-transpose, pooling
            outt = out_pool.tile([128, OUTW], F32, tag="outt")
            for g in range(NGRP) if "q" not in SKIP else []:
                c0 = 1600 * g
                qp = ps_q.tile([128, 400], F32)
                for j in range(4):
                    nc.tensor.matmul(qp[32 * j:32 * j + 32, :], q32,
                                     hh[:, c0 + 400 * j:c0 + 400 * j + 400],
                                     start=True, stop=True,
                                     tile_position=(0, 32 * j))
                wtile = w_pool.tile([128, 400], BF16, tag="wtile")
                nc.scalar.activation(wtile, qp, AF.Exp)

                if dbg and it == 0 and g == 0:
                    nc.gpsimd.dma_start(out=dbg_w.ap(), in_=wtile)

                # w strips -> [tlo, 4-strip dups] stationaries; within a
                # strip, cols are (b_off, t) with t natural; block k picks
                # (b_off = k//2, par = k%2) via a stride-2 t view.
                wt_ps = ps_wt.tile([TP, 512], BF16)
                for k in range(4):
                    nc.tensor.transpose(
                        wt_ps[:, 128 * k:128 * k + 128],
                        wtile[:, 100 * k:100 * k + 100], ident)
                wt_sb = wt_pool.tile([TP, 512], BF16, tag="wt_sb")
                nc.vector.tensor_copy(out=wt_sb, in_=wt_ps)

                if dbg and it == 0 and g == 0:
                    nc.gpsimd.dma_start(out=dbg_wt.ap(), in_=wt_sb)

                if "pool" in SKIP:
                    continue
                pp = ps_pool.tile([128, 258], F32)
                for b_off in range(2):
                    for j in range(4):
                        bb = 8 * g + 2 * j + b_off
                        for par in range(2):
                            k = 2 * b_off + par
                            nc.tensor.matmul(
                                pp[32 * j:32 * j + 32,
                                   128 * b_off:128 * b_off + 128],
                                wt_sb[:, 128 * k + 32 * j:128 * k + 32 * j + 32],
                                nt4[:, bb, par, :], start=par == 0,
                                stop=par == 1,
                                tile_position=(0, 32 * j))
                    for par in range(2):
                        k = 2 * b_off + par
                        nc.tensor.matmul(
                            pp[:, 256 + b_off:257 + b_off],
                            wt_sb[:, 128 * k:128 * k + 128],
                            ones1, start=par == 0, stop=par == 1)
                nc.vector.tensor_copy(
                    out=outt[:, 258 * g:258 * (g + 1)], in_=pp)

            if dbg and it == 0:
                nc.gpsimd.dma_start(out=dbg_nt.ap(), in_=nt)
                nc.gpsimd.dma_start(out=dbg_ht.ap(), in_=ht)
                nc.gpsimd.dma_start(out=dbg_hh.ap(), in_=hh)
                nc.gpsimd.dma_start(out=dbg_out.ap(), in_=outt)
            for j in range(4) if "pool" not in SKIP else []:
                nc.sync.dma_start(
                    out=out_dev.ap()[it, j, :],
                    in_=outt[32 * j:32 * j + 1, :],
                )

    return out_dev


def decode_out(arr, b_core=BC):
    """[nit, 4, OUTW] f32 -> pooled [b_core, D], wsum [b_core]."""
    nit = b_core // B_IT
    a = arr.reshape(nit, 4, NGRP, 258)
    # b = it*64 + 8*g + 2*j + b_off
    po = a[..., 0:256].reshape(nit, 4, NGRP, 2, D)
    po = np.transpose(po, (0, 2, 1, 3, 4)).reshape(b_core, D)
    dn = a[..., 256:258]
    dn = np.transpose(dn, (0, 2, 1, 3)).reshape(b_core)
    return po, dn


_cache = {}
LAST_RESULT = None
SKIP = set()
CFG = dict(nt=2, tp=1, mm=2, q=1, wt=1, pool=1, ht=2, hh=1, outt=2, wtb=4)


def _get_program(b_core):
    key = (b_core, tuple(sorted(SKIP)), tuple(sorted(CFG.items())))
    if key not in _cache:
        nc = bacc.Bacc("TRN2", target_bir_lowering=False, debug=False,
                       num_devices=NCORES)
        build(nc, b_core)
        nc.compile()
        _cache[key] = nc
    return _cache[key]


def kernel(**inputs):
    hist = np.ascontiguousarray(np.asarray(inputs["hist_embeddings"], np.float32))
    tgt = np.ascontiguousarray(np.asarray(inputs["target_embedding"], np.float32))
    W = np.ascontiguousarray(np.asarray(inputs["W_kernel"], np.float32))
    Wb = np.ascontiguousarray(np.asarray(inputs["W_bias"], np.float32))
    q = np.ascontiguousarray(np.asarray(inputs["q_kernel"], np.float32))
    # q_bias shifts every logit equally -> softmax-invariant -> ignored.

    nc = _get_program(BC)
    in_maps = []
    for c in range(NCORES):
        sl = slice(c * BC, (c + 1) * BC)
        in_maps.append({
            "hist": hist[sl], "target": tgt[sl],
            "W": W, "Wb": Wb, "q": q,
        })
    res = run_bass_kernel_spmd(nc, in_maps, core_ids=list(range(NCORES)))
    global LAST_RESULT
    LAST_RESULT = res
    outs = []
    for c in range(NCORES):
        pooled, wsum = decode_out(res.results[c]["out_dev"])
        outs.append(pooled / wsum[:, None])
    return np.concatenate(outs, axis=0).astype(np.float32)


def timed_run(inputs, iters=5, bcs=BC):
    """Device-resident repeated execution; returns (best_seconds, outputs)."""
    import time
    import jax
    from jax.sharding import Mesh, PartitionSpec
    from jax.experimental.shard_map import shard_map
    import concourse.mybir as mybir_
    from concourse.bass2jax import (install_neuronx_cc_hook, _bass_exec_p,
                                    partition_id_tensor)

    hist = np.ascontiguousarray(np.asarray(inputs["hist_embeddings"], np.float32))
    tgt = np.ascontiguousarray(np.asarray(inputs["target_embedding"], np.float32))
    W = np.ascontiguousarray(np.asarray(inputs["W_kernel"], np.float32))
    Wb = np.ascontiguousarray(np.asarray(inputs["W_bias"], np.float32))
    q = np.ascontiguousarray(np.asarray(inputs["q_kernel"], np.float32))
    hist = hist[:NCORES * bcs].reshape(NCORES, bcs, T, D).reshape(NCORES * bcs, T, D)
    tgt = tgt[:NCORES * bcs]
    nc = _get_program(bcs)
    install_neuronx_cc_hook()

    pid_name = nc.partition_id_tensor.name if nc.partition_id_tensor else None
    in_names, out_names, out_avals, zero_outs = [], [], [], []
    for alloc in nc.m.functions[0].allocations:
        if not isinstance(alloc, mybir_.MemoryLocationSet):
            continue
        name = alloc.memorylocations[0].name
        if alloc.kind == "ExternalInput":
            if name != pid_name:
                in_names.append(name)
        elif alloc.kind == "ExternalOutput":
            shape = tuple(alloc.tensor_shape)
            dtype = mybir_.dt.np(alloc.dtype)
            out_names.append(name)
            out_avals.append(jax.core.ShapedArray(shape, dtype))
            zero_outs.append(np.zeros(shape, dtype))
    all_names = in_names + out_names
    if pid_name is not None:
        all_names = all_names + [pid_name]

    import os
    chain = int(os.environ.get("KERNEL_CHAIN", "1"))

    def _body(*args):
        nin_ = len(in_names)
        ins_ = list(args[:nin_])
        outs = list(args[nin_:])
        for _ in range(chain):
            operands = ins_ + outs
            if pid_name is not None:
                operands = operands + [partition_id_tensor()]
            outs = list(_bass_exec_p.bind(
                *operands, out_avals=tuple(out_avals),
                in_names=tuple(all_names), out_names=tuple(out_names),
                lowering_input_output_aliases=(),
                sim_require_finite=True, sim_require_nnan=True, nc=nc))
        return tuple(outs)

    devices = jax.devices()[:NCORES]
    mesh = Mesh(np.array(devices), ("core",))
    nin = len(in_names) + len(out_names)
    fn = jax.jit(shard_map(_body, mesh=mesh,
                           in_specs=(PartitionSpec("core"),) * nin,
                           out_specs=(PartitionSpec("core"),) * len(out_names),
                           check_rep=False))
    full = {"hist": hist, "target": tgt,
            "W": np.concatenate([W] * NCORES, 0),
            "Wb": np.concatenate([Wb] * NCORES, 0),
            "q": np.concatenate([q] * NCORES, 0)}
    args = [full[n] for n in in_names] + [
        np.concatenate([z] * NCORES, 0) for z in zero_outs]
    sh = jax.sharding.NamedSharding(mesh, PartitionSpec("core"))
    dargs = [jax.device_put(a, sh) for a in args]
    res = fn(*dargs)
    jax.block_until_ready(res)
    import os
    pipeline = int(os.environ.get("KERNEL_PIPE", "1"))
    nin_ = len(in_names)
    best = float("inf")
    for _ in range(iters):
        t0 = time.perf_counter()
        r = tuple(dargs[nin_:])
        for _k in range(pipeline):
            r = fn(*dargs[:nin_], *r)
        jax.block_until_ready(r)
        best = min(best, time.perf_counter() - t0)
        res = r
    outs = [np.asarray(r) for r in res]
    per_core = np.split(outs[out_names.index("out_dev")], NCORES, axis=0)
    full_out = []
    for c in range(NCORES):
        pooled, wsum = decode_out(per_core[c], bcs)
        full_out.append(pooled / wsum[:, None])
    return best, np.concatenate(full_out, 0).astype(np.float32)


if __name__ == "__main__":
    rng = np.random.default_rng(0)
    ins = {
        "target_embedding": rng.standard_normal((B, D), dtype=np.float32),
        "hist_embeddings": rng.standard_normal((B, T, D), dtype=np.float32),
        "W_kernel": (rng.standard_normal((D, D), dtype=np.float32) / np.sqrt(D)),
        "W_bias": np.zeros(D, np.float32),
        "q_kernel": (rng.standard_normal((D, 1), dtype=np.float32) / np.sqrt(D)),
        "q_bias": np.zeros(1, np.float32),
    }
    out = kernel(**ins)
    print("out", out.shape, out.dtype)


# revision 19
# speedup vs baseline: 1.4899x; 1.4899x over previous
"""AttentionPooling Trainium2 kernel (v3: contiguous load + diagonal pooling).

Math (per batch row b):
    x   = target[b] + hist[b]              # [T, D]
    h   = relu(x @ W + Wb)                 # [T, D]
    lg  = h @ q (+ q_bias, softmax-invariant -> ignored)
    s   = softmax(lg)                      # over T
    out = sum_t s_t * hist[b, t]           # [D]

Device strategy (pure data parallel over batch across 8 cores).  Strided
HBM reads run at ~half bandwidth on real TRN2, so hist is loaded with a
single fully CONTIGUOUS fp32->bf16 cast DMA per 64-batch iteration into
the natural layout [p=(b,th), (tl,d)] (th = t//100, tl = t%100):
  - PE transposes the 100 [128,128] d-blocks -> xT [d, (tl, p)]; the
    PSUM->SBUF copy fuses the broadcast target add (packed APs keep the
    DVE 2x bf16 mode).
  - Main matmul: H^T = W^T @ xT (bf16, W stationary), relu+bias on ACT
    over 1024-col PSUM chunks.
  - q-matmul per tl-chunk: stationary = hh block [e,128], moving = q
    -> logits land NATURALLY as [p, tl] columns of one [128,100] PSUM
    tile; a single exp (ACT) with accum_out yields w AND the softmax
    denominators in one instruction.
  - Pooling: per tl one matmul, stationary = wdiag [128, 64] (w values
    scattered on the 2-diagonal (p, p//2), built by one DVE multiply
    with a constant 0/1 mask), moving = the natural hist block
    [128, 128] -> PSUM-accumulated [b, d] over all 100 tl.
  - Final normalize (divide by sum_t w) on host.
"""

import sys

sys.path.insert(0, "/opt/trn_rl_repo")

import numpy as np

import concourse.bacc as bacc
import concourse.bass as bass
import concourse.mybir as mybir
import concourse.tile as tile
from concourse import masks
from concourse.bass_utils import run_bass_kernel_spmd

F32 = mybir.dt.float32
BF16 = mybir.dt.bfloat16
AF = mybir.ActivationFunctionType

NCORES = 8
B, T, D = 16384, 200, 128
BC = B // NCORES          # 2048 batch rows per core
TL = 100                  # tl positions per partition (t = th*100 + tl)
B_IT = 64                 # batch rows per outer iteration
NC_IT = B_IT * T * D      # elements per iteration


def build(nc, b_core=BC, dbg=False):
    nit = b_core // B_IT
    hist = nc.dram_tensor("hist", [b_core, T, D], F32, kind="ExternalInput")
    tgt = nc.dram_tensor("target", [b_core, D], F32, kind="ExternalInput")
    w_in = nc.dram_tensor("W", [D, D], F32, kind="ExternalInput")
    wb_in = nc.dram_tensor("Wb", [D], F32, kind="ExternalInput")
    q_in = nc.dram_tensor("q", [D, 1], F32, kind="ExternalInput")
    out_pl = nc.dram_tensor("out_pl", [nit, B_IT, D], F32, kind="ExternalOutput")
    out_dn = nc.dram_tensor("out_dn", [nit, 128], F32, kind="ExternalOutput")
    if dbg:
        dbg_nt = nc.dram_tensor("dbg_nt", [128, TL * D], F32, kind="ExternalOutput")
        dbg_ht = nc.dram_tensor("dbg_ht", [128, B_IT * T], F32, kind="ExternalOutput")
        dbg_hh = nc.dram_tensor("dbg_hh", [128, B_IT * T], F32, kind="ExternalOutput")
        dbg_w = nc.dram_tensor("dbg_w", [128, TL], F32, kind="ExternalOutput")
        dbg_wd = nc.dram_tensor("dbg_wd", [128, TL * B_IT], F32, kind="ExternalOutput")

    from contextlib import ExitStack
    with tile.TileContext(nc) as tc, ExitStack() as es:
        consts = es.enter_context(tc.tile_pool(name="consts", bufs=1))
        nt_pool = es.enter_context(tc.tile_pool(name="nt", bufs=CFG["nt"]))
        ht_pool = es.enter_context(tc.tile_pool(name="ht", bufs=CFG["ht"]))
        h_pool = es.enter_context(tc.tile_pool(name="h", bufs=CFG["hh"]))
        w_pool = es.enter_context(tc.tile_pool(name="w", bufs=CFG["wb"]))
        out_pool = es.enter_context(tc.tile_pool(name="out", bufs=CFG["outt"]))
        ps_tp = es.enter_context(tc.tile_pool(name="ps_tp", bufs=CFG["tp"], space="PSUM"))
        ps_mm = es.enter_context(tc.tile_pool(name="ps_mm", bufs=CFG["mm"], space="PSUM"))
        ps_q = es.enter_context(tc.tile_pool(name="ps_q", bufs=CFG["q"], space="PSUM"))
        ps_pool = es.enter_context(tc.tile_pool(name="ps_pool", bufs=CFG["pool"], space="PSUM"))

        # ---- constants ----
        ident = consts.tile([128, 128], BF16)
        masks.make_identity(nc, ident[:, :])

        w_f32 = consts.tile([D, D], F32)
        nc.sync.dma_start(out=w_f32, in_=w_in.ap())
        w_bf = consts.tile([D, D], BF16)
        nc.vector.tensor_copy(out=w_bf, in_=w_f32)

        wbias = consts.tile([D, 1], F32)
        nc.sync.dma_start(out=wbias, in_=wb_in.ap()[:, None])

        q_f32 = consts.tile([D, 1], F32)
        nc.sync.dma_start(out=q_f32, in_=q_in.ap())
        q_bf = consts.tile([D, 1], BF16)
        nc.vector.tensor_copy(out=q_bf, in_=q_f32)

        # 2-diagonal mask: I2[p, b] = 1 if p // 2 == b else 0  [128, 64] bf16
        # built from the identity: I2[p, b] = ident[p, 2b] + ident[p, 2b+1]
        i2 = consts.tile([128, B_IT], BF16)
        idv = ident.rearrange("p (b u) -> p b u", u=2)
        nc.vector.tensor_add(i2, idv[:, :, 0], idv[:, :, 1])

        # targetT [d, b_core] bf16
        tgtT = consts.tile([D, b_core], BF16)
        for k in range((b_core + 127) // 128):
            bn = min(128, b_core - k * 128)
            t_f32 = w_pool.tile([128, D], F32, tag="tsetup")
            nc.sync.dma_start(out=t_f32[0:bn], in_=tgt.ap()[k * 128:k * 128 + bn, :])
            t_bf = w_pool.tile([128, D], BF16, tag="tsetup_bf")
            nc.vector.tensor_copy(out=t_bf[0:bn], in_=t_f32[0:bn])
            tpp = ps_tp.tile([128, 1024], BF16, tag="tp")
            nc.tensor.transpose(tpp[:, 0:bn], t_bf[0:bn], ident[0:bn, 0:bn])
            nc.vector.tensor_copy(out=tgtT[:, k * 128:k * 128 + bn], in_=tpp[:, 0:bn])

        # ---- main loop ----
        for it in range(nit):
            b0 = it * B_IT

            # natural tile: partition p=(b,th), free (tl, d); one contiguous
            # cast DMA for the whole 64-batch slice
            nt = nt_pool.tile([128, TL * D], BF16, tag="nt")
            src = hist.ap()[b0:b0 + B_IT]
            nc.gpsimd.dma_start(
                out=nt,
                in_=bass.AP(tensor=src.tensor, offset=src.offset,
                            ap=[[TL * D, 128], [1, TL * D]]),
            )

            # target expanded 2x: tgx2[d, p] = tgtT[d, b0 + p//2] -- i.e.
            # column index IS p = 2b+th, so the broadcast AP below has a
            # fully packed innermost dim (DVE 2x mode)
            tgx2 = w_pool.tile([128, B_IT * 2], BF16, tag="tgx2")
            sl = tgtT[:, b0:b0 + B_IT]
            nc.vector.tensor_copy(
                out=tgx2,
                in_=bass.AP(tensor=sl.tensor, offset=sl.offset,
                            ap=[sl.ap[0], sl.ap[1], [0, 2]]),
            )

            # transposes -> xT [d, (tl, p)] with fused target add
            ht = ht_pool.tile([128, B_IT * T], BF16, tag="ht")
            NTG = CFG["ntg"]          # transposes per PSUM group (8 -> 1 bank)
            for g in range((TL + NTG - 1) // NTG) if "tp" not in SKIP else []:
                t0 = NTG * g
                ng = min(NTG, TL - t0)
                tp = ps_tp.tile([128, NTG * 128], BF16, tag="tp")
                for u in range(ng):
                    nc.tensor.transpose(
                        tp[:, 128 * u:128 * u + 128],
                        nt[:, (t0 + u) * D:(t0 + u) * D + D], ident)
                nc.vector.tensor_add(
                    ht.rearrange("d (t p) -> d t p",
                                 p=128)[:, t0:t0 + ng, :],
                    tp.rearrange("d (t p) -> d t p", p=128)[:, 0:ng, :],
                    bass.AP(tensor=tgx2.tensor, offset=tgx2.offset,
                            ap=[tgx2.ap[0], [0, ng], [1, 128]]),
                )

            # H^T = relu(W^T xT + bias)  [e, (tl, p)]
            hh = h_pool.tile([128, B_IT * T], BF16, tag="hh")
            nmm = (B_IT * T) // 1024
            for k in range(nmm + 1) if "mm" not in SKIP else []:
                w_cols = 1024 if k < nmm else (B_IT * T) % 1024
                if w_cols == 0:
                    continue
                mm = ps_mm.tile([128, 1024], F32)
                for h in range((w_cols + 511) // 512):
                    c = 1024 * k + 512 * h
                    cw = min(512, w_cols - 512 * h)
                    nc.tensor.matmul(mm[:, 512 * h:512 * h + cw], w_bf,
                                     ht[:, c:c + cw], start=True, stop=True)
                nc.scalar.activation(hh[:, 1024 * k:1024 * k + w_cols],
                                     mm[:, 0:w_cols], AF.Relu, bias=wbias)

            # q-matmuls: logits land naturally [p, tl]
            qn = ps_q.tile([128, TL], F32)
            for tl in range(TL) if "q" not in SKIP else []:
                nc.tensor.matmul(qn[:, tl:tl + 1],
                                 hh[:, tl * 128:tl * 128 + 128], q_bf,
                                 start=True, stop=True)

            # one exp over all logits; accum gives softmax denominators
            wnat = w_pool.tile([128, TL], BF16, tag="wnat")
            dn_sb = out_pool.tile([128, 1], F32, tag="dn")
            if "q" not in SKIP:
                nc.scalar.activation(wnat, qn, AF.Exp, accum_out=dn_sb)
                nc.sync.dma_start(out=out_dn.ap()[it, :][:, None],
                                  in_=dn_sb)

            # wdiag build: wdiag[p, (tl, b)] = I2[p, b] * wnat[p, tl]
            wdiag = w_pool.tile([128, TL * B_IT], BF16, tag="wdiag")
            if "pool" not in SKIP:
                i2b = i2[:, :]
                wn = wnat[:, :]
                nc.vector.tensor_mul(
                    wdiag.rearrange("p (t b) -> p t b", b=B_IT),
                    bass.AP(tensor=i2b.tensor, offset=i2b.offset,
                            ap=[i2b.ap[0], [0, TL], i2b.ap[1]]),
                    bass.AP(tensor=wn.tensor, offset=wn.offset,
                            ap=[wn.ap[0], wn.ap[1], [0, B_IT]]),
                )

            # pooling: one matmul per tl, accumulated into [b, d]
            pl = ps_pool.tile([B_IT, D], F32)
            for tl in range(TL) if "pool" not in SKIP else []:
                nc.tensor.matmul(pl, wdiag[:, tl * B_IT:(tl + 1) * B_IT],
                                 nt[:, tl * D:tl * D + D],
                                 start=tl == 0, stop=tl == TL - 1)
            if "pool" not in SKIP:
                outt = out_pool.tile([B_IT, D], F32, tag="outt")
                nc.vector.tensor_copy(out=outt, in_=pl)
                nc.sync.dma_start(out=out_pl.ap()[it], in_=outt)

            if dbg and it == 0:
                nc.gpsimd.dma_start(out=dbg_nt.ap(), in_=nt)
                nc.gpsimd.dma_start(out=dbg_ht.ap(), in_=ht)
                nc.gpsimd.dma_start(out=dbg_hh.ap(), in_=hh)
                nc.gpsimd.dma_start(out=dbg_w.ap(), in_=wnat)
                nc.gpsimd.dma_start(out=dbg_wd.ap(), in_=wdiag)

    return out_pl


def decode_out(pl, dn, b_core=BC):
    """out_pl [nit, B_IT, D], out_dn [nit, 128] -> pooled, wsum per b."""
    nit = b_core // B_IT
    pooled = pl.reshape(b_core, D)
    d = dn.reshape(nit, B_IT, 2)
    wsum = (d[..., 0] + d[..., 1]).reshape(b_core)
    return pooled, wsum


_cache = {}
LAST_RESULT = None
SKIP = set()
CFG = dict(nt=2, tp=2, mm=2, q=1, pool=1, ht=2, hh=1, outt=2, wb=3, ntg=8)


def _get_program(b_core):
    key = (b_core, tuple(sorted(SKIP)), tuple(sorted(CFG.items())))
    if key not in _cache:
        nc = bacc.Bacc("TRN2", target_bir_lowering=False, debug=False,
                       num_devices=NCORES)
        build(nc, b_core)
        nc.compile()
        _cache[key] = nc
    return _cache[key]


def kernel(**inputs):
    hist = np.ascontiguousarray(np.asarray(inputs["hist_embeddings"], np.float32))
    tgt = np.ascontiguousarray(np.asarray(inputs["target_embedding"], np.float32))
    W = np.ascontiguousarray(np.asarray(inputs["W_kernel"], np.float32))
    Wb = np.ascontiguousarray(np.asarray(inputs["W_bias"], np.float32))
    q = np.ascontiguousarray(np.asarray(inputs["q_kernel"], np.float32))
    # q_bias shifts every logit equally -> softmax-invariant -> ignored.

    nc = _get_program(BC)
    in_maps = []
    for c in range(NCORES):
        sl = slice(c * BC, (c + 1) * BC)
        in_maps.append({
            "hist": hist[sl], "target": tgt[sl],
            "W": W, "Wb": Wb, "q": q,
        })
    res = run_bass_kernel_spmd(nc, in_maps, core_ids=list(range(NCORES)))
    global LAST_RESULT
    LAST_RESULT = res
    outs = []
    for c in range(NCORES):
        pooled, wsum = decode_out(res.results[c]["out_pl"],
                                  res.results[c]["out_dn"])
        outs.append(pooled / wsum[:, None])
    return np.concatenate(outs, axis=0).astype(np.float32)


def timed_run(inputs, iters=5, bcs=BC):
    """Device-resident repeated execution; returns (best_seconds, outputs)."""
    import time
    import jax
    from jax.sharding import Mesh, PartitionSpec
    from jax.experimental.shard_map import shard_map
    import concourse.mybir as mybir_
    from concourse.bass2jax import (install_neuronx_cc_hook, _bass_exec_p,
                                    partition_id_tensor)

    hist = np.ascontiguousarray(np.asarray(inputs["hist_embeddings"], np.float32))
    tgt = np.ascontiguousarray(np.asarray(inputs["target_embedding"], np.float32))
    W = np.ascontiguousarray(np.asarray(inputs["W_kernel"], np.float32))
    Wb = np.ascontiguousarray(np.asarray(inputs["W_bias"], np.float32))
    q = np.ascontiguousarray(np.asarray(inputs["q_kernel"], np.float32))
    hist = hist[:NCORES * bcs].reshape(NCORES * bcs, T, D)
    tgt = tgt[:NCORES * bcs]
    nc = _get_program(bcs)
    install_neuronx_cc_hook()

    pid_name = nc.partition_id_tensor.name if nc.partition_id_tensor else None
    in_names, out_names, out_avals, zero_outs = [], [], [], []
    for alloc in nc.m.functions[0].allocations:
        if not isinstance(alloc, mybir_.MemoryLocationSet):
            continue
        name = alloc.memorylocations[0].name
        if alloc.kind == "ExternalInput":
            if name != pid_name:
                in_names.append(name)
        elif alloc.kind == "ExternalOutput":
            shape = tuple(alloc.tensor_shape)
            dtype = mybir_.dt.np(alloc.dtype)
            out_names.append(name)
            out_avals.append(jax.core.ShapedArray(shape, dtype))
            zero_outs.append(np.zeros(shape, dtype))
    all_names = in_names + out_names
    if pid_name is not None:
        all_names = all_names + [pid_name]

    import os
    chain = int(os.environ.get("KERNEL_CHAIN", "1"))

    def _body(*args):
        nin_ = len(in_names)
        ins_ = list(args[:nin_])
        outs = list(args[nin_:])
        for _ in range(chain):
            operands = ins_ + outs
            if pid_name is not None:
                operands = operands + [partition_id_tensor()]
            outs = list(_bass_exec_p.bind(
                *operands, out_avals=tuple(out_avals),
                in_names=tuple(all_names), out_names=tuple(out_names),
                lowering_input_output_aliases=(),
                sim_require_finite=True, sim_require_nnan=True, nc=nc))
        return tuple(outs)

    devices = jax.devices()[:NCORES]
    mesh = Mesh(np.array(devices), ("core",))
    nin = len(in_names) + len(out_names)
    fn = jax.jit(shard_map(_body, mesh=mesh,
                           in_specs=(PartitionSpec("core"),) * nin,
                           out_specs=(PartitionSpec("core"),) * len(out_names),
                           check_rep=False))
    full = {"hist": hist, "target": tgt,
            "W": np.concatenate([W] * NCORES, 0),
            "Wb": np.concatenate([Wb] * NCORES, 0),
            "q": np.concatenate([q] * NCORES, 0)}
    args = [full[n] for n in in_names] + [
        np.concatenate([z] * NCORES, 0) for z in zero_outs]
    sh = jax.sharding.NamedSharding(mesh, PartitionSpec("core"))
    dargs = [jax.device_put(a, sh) for a in args]
    res = fn(*dargs)
    jax.block_until_ready(res)
    import os
    pipeline = int(os.environ.get("KERNEL_PIPE", "1"))
    nin_ = len(in_names)
    best = float("inf")
    for _ in range(iters):
        t0 = time.perf_counter()
        r = tuple(dargs[nin_:])
        for _k in range(pipeline):
            r = fn(*dargs[:nin_], *r)
        jax.block_until_ready(r)
        best = min(best, time.perf_counter() - t0)
        res = r
    outs = [np.asarray(r) for r in res]
    pl_all = np.split(outs[out_names.index("out_pl")], NCORES, axis=0)
    dn_all = np.split(outs[out_names.index("out_dn")], NCORES, axis=0)
    full_out = []
    for c in range(NCORES):
        pooled, wsum = decode_out(pl_all[c], dn_all[c], bcs)
        full_out.append(pooled / wsum[:, None])
    return best, np.concatenate(full_out, 0).astype(np.float32)


if __name__ == "__main__":
    rng = np.random.default_rng(0)
    ins = {
        "target_embedding": rng.standard_normal((B, D), dtype=np.float32),
        "hist_embeddings": rng.standard_normal((B, T, D), dtype=np.float32),
        "W_kernel": (rng.standard_normal((D, D), dtype=np.float32) / np.sqrt(D)),
        "W_bias": np.zeros(D, np.float32),
        "q_kernel": (rng.standard_normal((D, 1), dtype=np.float32) / np.sqrt(D)),
        "q_bias": np.zeros(1, np.float32),
    }
    out = kernel(**ins)
    print("out", out.shape, out.dtype)
